# revision 1
# baseline (speedup 1.0000x reference)
"""Trainium2 Bass kernel for BLOOM attention block (nn_BloomAttention).

Self-contained: builds an SPMD Bass/Tile kernel for 8 NeuronCores.
Heads are sharded (tensor parallel) for QKV projection + attention;
an AllToAll then redistributes context to a sequence-sharded layout
for the dense projection + residual, so each core emits a disjoint
slice of rows and the host only concatenates.

kernel(**inputs) takes the FULL unsharded inputs and returns the FULL
output [B, S, H] float32.

Numerics notes:
- attention_mask input is ignored: the reference always builds the
  standard causal mask, which we implement exactly (masked probs are
  exactly 0 after exp of -1e30, matching softmax+mask-multiply).
- softmax uses the exact shift s - slope*i (a per-row constant, the
  alibi at the diagonal) instead of the row max; mathematically
  identical and bounded: |qk|/sqrt(128) <= 11.4 so exp never overflows.
"""

import math
from contextlib import ExitStack
from dataclasses import dataclass

import numpy as np

import concourse.bass as bass
import concourse.bacc as bacc
import concourse.mybir as mybir
import concourse.tile as tile
from concourse.masks import make_identity, make_causal_mask

F32 = mybir.dt.float32
F32R = mybir.dt.float32r
BF16 = mybir.dt.bfloat16
AF = mybir.ActivationFunctionType
ALU = mybir.AluOpType

NEG_BIG = -1e30


@dataclass(frozen=True)
class Cfg:
    B: int = 2
    S: int = 2048
    H: int = 2048
    NH: int = 16
    n_cores: int = 8

    @property
    def HD(self):
        return self.H // self.NH

    @property
    def hpc(self):
        return self.NH // self.n_cores

    @property
    def rows(self):
        return self.B * self.S

    @property
    def shard(self):
        return self.rows // self.n_cores

    @property
    def wcols(self):
        return self.hpc * 3 * self.HD

    @property
    def norm(self):
        return math.sqrt(self.HD)


DEFAULT_CFG = Cfg()


def _cdiv(a, b):
    return (a + b - 1) // b


def build_nc(cfg: Cfg = DEFAULT_CFG):
    """Build the SPMD Bass module (same program on every core)."""
    P = 128
    KT = cfg.H // P            # contraction tiles over H (== NH)
    QT = cfg.S // P            # q tiles per (b, h) pair
    RC = 512                   # projection row-chunk
    NRC = cfg.rows // RC
    M = cfg.wcols // P         # qkv out col tiles per core (hpc*3)
    SCHUNK = 512               # scores psum chunk width (k columns)
    assert cfg.HD == 128

    nc = bacc.Bacc(
        "TRN2",
        target_bir_lowering=False,
        debug=False,
        num_devices=cfg.n_cores,
    )

    # ---- DRAM I/O (per-core shards prepared host-side) ----
    hid_d = nc.dram_tensor("hid", [cfg.rows, cfg.H], F32, kind="ExternalInput").ap()
    wqkv_d = nc.dram_tensor("wqkv", [cfg.H, cfg.wcols], F32, kind="ExternalInput").ap()
    bqkv_d = nc.dram_tensor("bqkv", [1, cfg.wcols], F32, kind="ExternalInput").ap()
    alibi_d = nc.dram_tensor("alibi", [cfg.hpc, cfg.S], F32, kind="ExternalInput").ap()
    wd_d = nc.dram_tensor("wd", [cfg.H, cfg.H], F32, kind="ExternalInput").ap()  # pre-transposed: [in, out]
    bd_d = nc.dram_tensor("bd", [1, cfg.H], F32, kind="ExternalInput").ap()
    res_d = nc.dram_tensor("res", [cfg.shard, cfg.H], F32, kind="ExternalInput").ap()
    out_d = nc.dram_tensor("out", [cfg.shard, cfg.H], F32, kind="ExternalOutput").ap()

    a2a_in = [
        nc.dram_tensor(f"a2a_in{h}", [cfg.n_cores, P, cfg.shard], BF16).ap()
        for h in range(cfg.hpc)
    ]
    a2a_out = [
        nc.dram_tensor(f"a2a_out{h}", [cfg.n_cores, P, cfg.shard], BF16).ap()
        for h in range(cfg.hpc)
    ]

    with tile.TileContext(nc, num_cores=cfg.n_cores) as tc, ExitStack() as ctx:
        const = ctx.enter_context(tc.tile_pool(name="const", bufs=1))

        ident = const.tile([P, P], BF16, tag="ident")
        make_identity(nc, ident[:])
        # additive causal block: [q, k] = 0 if k <= q else NEG_BIG
        causal = const.tile([P, P], BF16, tag="causal")
        make_causal_mask(nc, causal[:], mask_val=NEG_BIG)

        ones_row = const.tile([1, RC], BF16, tag="ones")
        nc.vector.memset(ones_row[:], 1.0)
        ones3 = const.tile([3, P], BF16, tag="ones3")
        nc.vector.memset(ones3[:], 1.0)

        bqkv_sb = const.tile([1, cfg.wcols], BF16, tag="bqkv")
        nc.gpsimd.dma_start(bqkv_sb[:], bqkv_d)  # cast f32->bf16
        bd_sb = const.tile([1, cfg.H], BF16, tag="bd")
        nc.gpsimd.dma_start(bd_sb[:], bd_d)  # cast

        # per-head alibi: 3-way bf16 split rows (summed exactly by a
        # K=3 matmul against ones3) + negated f32 per-partition columns
        arow3, nacol = [], []
        with tc.tile_pool(name="alibi_tmp", bufs=2) as atmp:
            for h in range(cfg.hpc):
                rf = atmp.tile([1, cfg.S], F32, tag="arowf")
                nc.sync.dma_start(rf[:], alibi_d[h : h + 1, :])
                a3 = const.tile([3, cfg.S], BF16, tag=f"arow3{h}")
                err1 = atmp.tile([1, cfg.S], F32, tag="aerr1")
                err2 = atmp.tile([1, cfg.S], F32, tag="aerr2")
                hi = atmp.tile([1, cfg.S], BF16, tag="ahi")
                lo = atmp.tile([1, cfg.S], BF16, tag="alo")
                lo2 = atmp.tile([1, cfg.S], BF16, tag="alo2")
                nc.vector.tensor_copy(hi[:], rf[:])
                nc.vector.tensor_tensor(err1[:], rf[:], hi[:], op=ALU.subtract)
                nc.vector.tensor_copy(lo[:], err1[:])
                nc.vector.tensor_tensor(err2[:], err1[:], lo[:], op=ALU.subtract)
                nc.vector.tensor_copy(lo2[:], err2[:])
                # assemble [3, S] via DMA (engines can't write partition 1/2)
                nc.sync.dma_start(a3[0:1, :], hi[:])
                nc.sync.dma_start(a3[1:2, :], lo[:])
                nc.sync.dma_start(a3[2:3, :], lo2[:])
                arow3.append(a3)
                c_ = const.tile([P, QT], F32, tag=f"nacol{h}")
                nc.sync.dma_start(c_[:], alibi_d[h].rearrange("(t p) -> p t", p=P))
                nc.vector.tensor_scalar_mul(c_[:], c_[:], -1.0)
                nacol.append(c_)

        with tc.tile_pool(name="qkv", bufs=1) as qkv_pool, tc.tile_pool(
            name="ctxT", bufs=1
        ) as ctxT_pool:
            fusedT = [
                qkv_pool.tile([P, cfg.rows], BF16, tag=f"fusedT{m}", name=f"fusedT{m}") for m in range(M)
            ]
            qT = lambda h: fusedT[3 * h + 0]
            kTt = lambda h: fusedT[3 * h + 1]
            vT = lambda h: fusedT[3 * h + 2]
            ctxT = [
                ctxT_pool.tile([P, cfg.rows], BF16, tag=f"ctxT{h}", name=f"ctxT{h}")
                for h in range(cfg.hpc)
            ]

            # ====== Phase W+1: W_qkv transpose, then fused QKV projection ======
            with tc.tile_pool(name="wqT", bufs=1) as wq_pool:
                wqkvT = [
                    wq_pool.tile([P, cfg.wcols], BF16, tag=f"wqkvT{k}", name=f"wqkvT{k}")
                    for k in range(KT)
                ]


                with tc.tile_pool(name="hnat", bufs=5) as hnat_pool, tc.tile_pool(
                    name="hidT", bufs=2
                ) as hidT_pool, tc.tile_pool(
                    name="wqf", bufs=2
                ) as wqf_pool, tc.tile_pool(
                    name="fpsum", bufs=1, space="PSUM"
                ) as fpsum_pool:
                    for k in range(KT):
                        wf = wqf_pool.tile([P, cfg.wcols], F32, tag="wqf")
                        nc.sync.dma_start(wf[:], wqkv_d[k * P : (k + 1) * P, :])
                        nc.vector.tensor_copy(wqkvT[k][:], wf[:])
                    for rc in range(NRC):
                        nat = []
                        for j in range(RC // P):
                            t_ = hnat_pool.tile([P, cfg.H], BF16, tag="hnat")
                            r0 = rc * RC + j * P
                            nc.gpsimd.dma_start(t_[:], hid_d[r0 : r0 + P, :])
                            nat.append(t_)
                        hidTbuf = hidT_pool.tile([P, KT * RC], BF16, tag="hidT")
                        hidTv = hidTbuf[:].rearrange("p (k rc) -> p k rc", k=KT)
                        for j in range(RC // P):
                            nc.sync.dma_start(
                                hidTv[:, :, j * P : (j + 1) * P],
                                nat[j][:],
                                transpose=True,
                            )
                        hidT = [
                            hidTbuf[:, k * RC : (k + 1) * RC] for k in range(KT)
                        ]
                        for m in range(M):
                            fp = fpsum_pool.tile([P, RC], F32, tag=f"fp{m}")
                            nc.tensor.matmul(
                                fp[:],
                                bqkv_sb[:, m * P : (m + 1) * P],
                                ones_row[:],
                                start=True,
                                stop=False,
                            )
                            for k in range(KT):
                                nc.tensor.matmul(
                                    fp[:],
                                    wqkvT[k][:, m * P : (m + 1) * P],
                                    hidT[k],
                                    start=False,
                                    stop=(k == KT - 1),
                                )
                            scale = (1.0 / cfg.norm) if (m % 3 == 0) else 1.0
                            nc.scalar.activation(
                                fusedT[m][:, rc * RC : (rc + 1) * RC],
                                fp[:],
                                AF.Copy,
                                scale=scale,
                            )

            # ====== Phase 2: attention per (b, head) ======
            with tc.tile_pool(name="att_sb", bufs=3) as att_sb, tc.tile_pool(
                name="probsT", bufs=1
            ) as pT_pool, tc.tile_pool(name="vnat", bufs=2) as v_pool, tc.tile_pool(
                name="den", bufs=4 * QT
            ) as den_pool, tc.tile_pool(
                name="scp", bufs=4, space="PSUM"
            ) as sc_pool, tc.tile_pool(
                name="xp", bufs=4, space="PSUM"
            ) as xp_pool:
                for h in range(cfg.hpc):
                    for b in range(cfg.B):
                        base = b * cfg.S
                        # v natural [k-part, hd] from vT via PE transpose
                        v_sb = v_pool.tile([P, cfg.S], BF16, tag="vnat")
                        for g in range(QT // 4):
                            tp = xp_pool.tile([P, 512], BF16, tag="xp")
                            for j in range(4):
                                kt = 4 * g + j
                                nc.tensor.transpose(
                                    tp[:, j * P : (j + 1) * P],
                                    vT(h)[:, base + kt * P : base + (kt + 1) * P],
                                    ident[:],
                                )
                            nc.vector.tensor_copy(
                                v_sb[:, g * 512 : (g + 1) * 512], tp[:]
                            )

                        probsT = [
                            pT_pool.tile([P, (t + 1) * P], BF16, tag=f"pT{t}", name=f"pT{t}")
                            for t in range(QT)
                        ]

                        for t in range(QT):
                            live = (t + 1) * P
                            probs_f = att_sb.tile([P, live], F32, tag="probs_f")
                            dens = []
                            for c0 in range(0, live, SCHUNK):
                                cw = min(SCHUNK, live - c0)
                                sc = sc_pool.tile([P, SCHUNK], F32, tag="sc")
                                for n0 in range(0, cw, 512):
                                    nw = min(512, cw - n0)
                                    last = c0 + n0 + nw == live
                                    nc.tensor.matmul(
                                        sc[:, n0 : n0 + nw],
                                        qT(h)[:, base + t * P : base + (t + 1) * P],
                                        kTt(h)[
                                            :,
                                            base + c0 + n0 : base + c0 + n0 + nw,
                                        ],
                                        start=True,
                                        stop=False,
                                    )
                                    nc.tensor.matmul(
                                        sc[:, n0 : n0 + nw],
                                        ones3[:],
                                        arow3[h][:, c0 + n0 : c0 + n0 + nw],
                                        start=False,
                                        stop=not last,
                                    )
                                    if last:
                                        nc.tensor.matmul(
                                            sc[:, n0 + nw - P : n0 + nw],
                                            ident[:],
                                            causal[:],
                                            start=False,
                                            stop=True,
                                        )
                                den = den_pool.tile([P, 1], F32, tag="den")
                                nc.scalar.activation(
                                    probs_f[:, c0 : c0 + cw],
                                    sc[:, :cw],
                                    AF.Exp,
                                    bias=nacol[h][:, t : t + 1],
                                    accum_out=den[:],
                                )
                                dens.append(den)
                            den = dens[0]
                            for dd in dens[1:]:
                                nc.vector.tensor_tensor(
                                    den[:], den[:], dd[:], op=ALU.add
                                )
                            rden = den_pool.tile([P, 1], F32, tag="rden")
                            nc.vector.reciprocal(rden[:], den[:])
                            probs_n = att_sb.tile([P, live], BF16, tag="probs_n")
                            nc.vector.tensor_scalar(
                                probs_n[:], probs_f[:], rden[:], None, op0=ALU.mult
                            )
                            for g in range(_cdiv(t + 1, 4)):
                                gw = min(4, t + 1 - 4 * g)
                                tp = xp_pool.tile([P, 512], BF16, tag="xp")
                                for j in range(gw):
                                    kt = 4 * g + j
                                    nc.tensor.transpose(
                                        tp[:, j * P : (j + 1) * P],
                                        probs_n[:, kt * P : (kt + 1) * P],
                                        ident[:],
                                    )
                                nc.vector.tensor_copy(
                                    probsT[t][:, 4 * g * P : (4 * g + gw) * P],
                                    tp[:, : gw * P],
                                )

                        # PV: contextT[hd, q] accumulated over k tiles
                        for qb in range(QT * P // 512):
                            cx = xp_pool.tile([P, 512], F32, tag="xp")
                            tlo, thi = 4 * qb, min(4 * qb + 4, QT)
                            for kt in range(thi):
                                for t in range(max(kt, tlo), thi):
                                    nc.tensor.matmul(
                                        cx[:, (t - tlo) * P : (t - tlo + 1) * P],
                                        v_sb[:, kt * P : (kt + 1) * P],
                                        probsT[t][:, kt * P : (kt + 1) * P],
                                        start=(kt == 0 and t == tlo),
                                        stop=(kt == thi - 1 and t == thi - 1),
                                    )
                            nc.vector.tensor_copy(
                                ctxT[h][:, base + qb * 512 : base + (qb + 1) * 512],
                                cx[:],
                            )
                    # stage + AllToAll for this head (overlaps next head)
                    if b == cfg.B - 1:
                        for j in range(cfg.n_cores):
                            nc.sync.dma_start(
                                a2a_in[h][j],
                                ctxT[h][:, j * cfg.shard : (j + 1) * cfg.shard],
                            )
                        nc.gpsimd.collective_compute(
                            "AllToAll",
                            ALU.bypass,
                            replica_groups=[list(range(cfg.n_cores))],
                            ins=[a2a_in[h].opt()],
                            outs=[a2a_out[h].opt()],
                        )


        # ====== Phase 4: dense + residual (sequence-sharded) ======
        with tc.tile_pool(name="wdT", bufs=1) as wdT_pool, tc.tile_pool(
            name="ctxf", bufs=1
        ) as ctxf_pool, tc.tile_pool(name="dsb", bufs=2) as dsb_pool, tc.tile_pool(
            name="dpsum", bufs=2, space="PSUM"
        ) as dp_pool:
            wdT = [wdT_pool.tile([P, cfg.H], BF16, tag=f"wdT{k}", name=f"wdT{k}") for k in range(KT)]
            for k in range(KT):
                nc.gpsimd.dma_start(wdT[k][:], wd_d[k * P : (k + 1) * P, :])  # cast
            korder = [
                g
                for h in range(cfg.hpc)
                for g in range(cfg.NH)
                if g % cfg.hpc == h
            ]
            ctxf = {}
            for g in korder:
                t_ = ctxf_pool.tile([P, cfg.shard], BF16, tag=f"ctxf{g}", name=f"ctxf{g}")
                nc.sync.dma_start(t_[:], a2a_out[g % cfg.hpc][g // cfg.hpc])
                ctxf[g] = t_

            for m in range(cfg.shard // P):
                dp = dp_pool.tile([P, cfg.H], F32, tag="dp")
                for nb in range(cfg.H // 512):
                    nc.tensor.matmul(
                        dp[:, nb * 512 : (nb + 1) * 512],
                        ones_row[:, :P],
                        bd_sb[:, nb * 512 : (nb + 1) * 512],
                        start=True,
                        stop=False,
                    )
                for ki, k in enumerate(korder):
                    for nb in range(cfg.H // 512):
                        nc.tensor.matmul(
                            dp[:, nb * 512 : (nb + 1) * 512],
                            ctxf[k][:, m * P : (m + 1) * P],
                            wdT[k][:, nb * 512 : (nb + 1) * 512],
                            start=False,
                            stop=(ki == len(korder) - 1),
                        )
                res_sb = dsb_pool.tile([P, cfg.H], F32, tag="res")
                nc.sync.dma_start(res_sb[:], res_d[m * P : (m + 1) * P, :])
                out_sb = dsb_pool.tile([P, cfg.H], F32, tag="outsb")
                nc.vector.tensor_tensor(out_sb[:], dp[:], res_sb[:], op=ALU.add)
                nc.sync.dma_start(out_d[m * P : (m + 1) * P, :], out_sb[:])

    nc.compile()
    return nc


def make_in_maps(inputs: dict, cfg: Cfg = DEFAULT_CFG):
    """Shard the full inputs into per-core input maps."""
    hs = np.ascontiguousarray(
        np.asarray(inputs["hidden_states"], dtype=np.float32).reshape(cfg.rows, cfg.H)
    )
    res = np.asarray(inputs["residual"], dtype=np.float32).reshape(cfg.rows, cfg.H)
    wqkv = np.asarray(inputs["W_qkv"], dtype=np.float32)
    bqkv = np.asarray(inputs["b_qkv"], dtype=np.float32)
    wd = np.ascontiguousarray(np.asarray(inputs["W_dense"], dtype=np.float32).T)
    bd = np.ascontiguousarray(
        np.asarray(inputs["b_dense"], dtype=np.float32).reshape(1, cfg.H)
    )
    alibi = np.asarray(inputs["alibi"], dtype=np.float32).reshape(cfg.B, cfg.NH, cfg.S)

    in_maps = []
    for c in range(cfg.n_cores):
        w0 = c * cfg.wcols
        in_maps.append(
            {
                "hid": hs,
                "wqkv": np.ascontiguousarray(wqkv[w0 : w0 + cfg.wcols].T),
                "bqkv": np.ascontiguousarray(bqkv[w0 : w0 + cfg.wcols].reshape(1, -1)),
                "alibi": np.ascontiguousarray(alibi[0, c * cfg.hpc : (c + 1) * cfg.hpc]),
                "wd": wd,
                "bd": bd,
                "res": np.ascontiguousarray(res[c * cfg.shard : (c + 1) * cfg.shard]),
            }
        )
    return in_maps


def assemble_out(results, cfg: Cfg = DEFAULT_CFG) -> np.ndarray:
    out = np.concatenate([results[c]["out"] for c in range(cfg.n_cores)], axis=0)
    return np.ascontiguousarray(out.reshape(cfg.B, cfg.S, cfg.H).astype(np.float32))


_NC_CACHE = {}


def get_nc(cfg: Cfg = DEFAULT_CFG):
    if cfg not in _NC_CACHE:
        _NC_CACHE[cfg] = build_nc(cfg)
    return _NC_CACHE[cfg]


def kernel(**inputs) -> np.ndarray:
    from concourse.bass_utils import run_bass_kernel_spmd

    cfg = DEFAULT_CFG
    nc = get_nc(cfg)
    in_maps = make_in_maps(inputs, cfg)
    r = run_bass_kernel_spmd(nc, in_maps, core_ids=list(range(cfg.n_cores)))
    return assemble_out(r.results, cfg)



# revision 6
# speedup vs baseline: 1.6282x; 1.6282x over previous
"""Trainium2 Bass kernel for BLOOM attention block (nn_BloomAttention).

Self-contained SPMD Bass/Tile kernel for 8 NeuronCores; heads are
tensor-parallel (2 per core), an AllToAll redistributes context to a
sequence-sharded layout for the dense projection + residual.

kernel(**inputs) takes the FULL unsharded inputs and returns the FULL
output [B, S, H] float32.

Key structure (v2):
- Host pre-transposes hidden to [H, rows] and pre-casts all weights to
  bf16, so the QKV projection is pure matmul (no on-chip transposes).
- Attention computes scores TRANSPOSED (scoresT[k, q]) - both operands
  already live in [hd, row] layout - then exp(score) is multiplied by a
  precomputed ALiBi decay table F[k, q] = exp(slope*(k-q)) whose zeros
  also implement the causal mask.  softmax becomes exact with the
  implicit shift slope*q which never needs to be materialized.
- PV uses the probs block as the matmul stationary against V augmented
  with a ones column, producing context in natural [q, hd] layout plus
  the softmax denominator for free; normalization is then a cheap
  per-partition scale.
- Far off-diagonal blocks whose ALiBi decay underflows (< 1e-8 relative)
  are skipped entirely; heads are assigned to cores as {c, c+8} so the
  skip pattern is uniform across cores (same SPMD program).
- Heavy (low-slope) heads run first so their AllToAll overlaps the
  light heads' attention; the dense projection runs as two passes so
  the second AllToAll overlaps the first pass.
"""

import math
from contextlib import ExitStack
from dataclasses import dataclass

import numpy as np
import ml_dtypes

import concourse.bass as bass
import concourse.bacc as bacc
import concourse.mybir as mybir
import concourse.tile as tile
from concourse.masks import make_identity

F32 = mybir.dt.float32
BF16 = mybir.dt.bfloat16
AF = mybir.ActivationFunctionType
ALU = mybir.AluOpType

BF16NP = ml_dtypes.bfloat16
# drop a 128-block diagonal d when slope*(128d - 127) > LOGDROP
# (relative prob weight < exp(2*smax - LOGDROP) ~ 1e-10..1e-8)
LOGDROP = 46.0


@dataclass(frozen=True)
class Cfg:
    B: int = 2
    S: int = 2048
    H: int = 2048
    NH: int = 16
    n_cores: int = 8

    @property
    def HD(self):
        return self.H // self.NH

    @property
    def rows(self):
        return self.B * self.S

    @property
    def shard(self):
        return self.rows // self.n_cores

    @property
    def wcols(self):
        return 2 * 3 * self.HD

    @property
    def norm(self):
        return math.sqrt(self.HD)


DEFAULT_CFG = Cfg()
P = 128


def _cdiv(a, b):
    return (a + b - 1) // b


def slope_to_D(slope: float) -> int:
    """Max diagonal-block offset d that still carries weight for a head."""
    if slope <= 0.0:
        return 15
    return min(15, int((LOGDROP / slope + 127.0) // 128.0))


def build_nc(d_pair=(15, 6), cfg: Cfg = DEFAULT_CFG):
    """Build the SPMD Bass module (same program on every core).

    d_pair = (D of slot0/heavy heads 8..15, D of slot1/light heads 0..7):
    per q-tile t, only k-tiles kt in [t-D, t] are computed.
    """
    QT = cfg.S // P            # 16 q/k tiles per (b, slot)
    KT = cfg.H // P            # 16 contraction tiles over H
    RC = 1024                  # projection row-chunk
    NRC = cfg.rows // RC
    M = 6                      # qkv out col tiles per core (2 slots x q,k,v)
    VW = 132                   # v_aug per-ktile stride: 128 v cols + ones + pad
    assert cfg.HD == P

    nc = bacc.Bacc(
        "TRN2",
        target_bir_lowering=False,
        debug=False,
        num_devices=cfg.n_cores,
    )

    # ---- DRAM I/O (per-core shards prepared host-side, all pre-cast) ----
    hidT_d = nc.dram_tensor("hidT", [cfg.H, cfg.rows], BF16, kind="ExternalInput").ap()
    wqkvT_d = nc.dram_tensor("wqkvT", [cfg.H, cfg.wcols], BF16, kind="ExternalInput").ap()
    bq_d = nc.dram_tensor("bq", [P, M], F32, kind="ExternalInput").ap()
    fcat_d = nc.dram_tensor("fcat", [2, P, cfg.S], BF16, kind="ExternalInput").ap()
    wd_d = nc.dram_tensor("wd", [cfg.H, cfg.H], BF16, kind="ExternalInput").ap()
    res_d = nc.dram_tensor("res", [cfg.shard, cfg.H], F32, kind="ExternalInput").ap()
    out_d = nc.dram_tensor("out", [cfg.shard, cfg.H], F32, kind="ExternalOutput").ap()

    a2a_in = [
        nc.dram_tensor(f"a2a_in{s}", [cfg.n_cores, P, cfg.shard], BF16).ap()
        for s in range(2)
    ]
    a2a_out = [
        nc.dram_tensor(f"a2a_out{s}", [cfg.n_cores, P, cfg.shard], BF16).ap()
        for s in range(2)
    ]

    with tile.TileContext(nc, num_cores=cfg.n_cores) as tc, ExitStack() as ctx:
        const = ctx.enter_context(tc.tile_pool(name="const", bufs=1))

        ident = const.tile([P, P], BF16, tag="ident")
        make_identity(nc, ident[:])
        bq_sb = const.tile([P, M], F32, tag="bq")
        nc.sync.dma_start(bq_sb[:], bq_d)
        # per-slot alibi decay tables; slot1 only needs d <= D1
        fcat_sb = []
        for s in range(2):
            cols = min(QT, d_pair[s] + 1) * P
            f_ = const.tile([P, cols], BF16, tag=f"fcat{s}", name=f"fcat{s}")
            nc.sync.dma_start(f_[:], fcat_d[s][:, :cols])
            fcat_sb.append(f_)

        ctxT_pool = ctx.enter_context(tc.tile_pool(name="ctxT", bufs=1))
        wdh_pool = ctx.enter_context(tc.tile_pool(name="wdh", bufs=1))
        fused_ctx = ExitStack()
        fused_pool = fused_ctx.enter_context(tc.tile_pool(name="fused", bufs=1))

        fusedT = [
            fused_pool.tile([P, cfg.rows], BF16, tag=f"fusedT{m}", name=f"fusedT{m}")
            for m in range(M)
        ]
        qT = lambda s: fusedT[3 * s + 0]
        kTt = lambda s: fusedT[3 * s + 1]
        vT = lambda s: fusedT[3 * s + 2]
        ctxT = [
            ctxT_pool.tile([P, cfg.rows], BF16, tag=f"ctxT{s}", name=f"ctxT{s}")
            for s in range(2)
        ]
        # W_dense rows for the heavy heads (8..15), prefetched in phase 1
        wdT = {}
        for g in range(8, 16):
            wdT[g] = wdh_pool.tile([P, cfg.H], BF16, tag=f"wdT{g}", name=f"wdT{g}")

        # ====== Phase 1: fused QKV projection ======
        with tc.tile_pool(name="wq", bufs=1) as wq_pool, tc.tile_pool(
            name="hid", bufs=1
        ) as hid_pool, tc.tile_pool(name="fp", bufs=2, space="PSUM") as fp_pool:
            wqkvT = [
                wq_pool.tile([P, cfg.wcols], BF16, tag=f"wqkvT{k}", name=f"wqkvT{k}")
                for k in range(KT)
            ]
            for k in range(KT):
                nc.sync.dma_start(wqkvT[k][:], wqkvT_d[k * P : (k + 1) * P, :])
            for rc in range(NRC):
                hids = []
                for k in range(KT):
                    t_ = hid_pool.tile([P, RC], BF16, tag=f"hid{k}", name=f"hid{k}")
                    nc.sync.dma_start(
                        t_[:], hidT_d[k * P : (k + 1) * P, rc * RC : (rc + 1) * RC]
                    )
                    hids.append(t_)
                if rc == 0:
                    # prefetch heavy-head dense weights behind the projection
                    for g in range(8, 16):
                        nc.sync.dma_start(wdT[g][:], wd_d[g * P : (g + 1) * P, :])
                for m in range(M):
                    fp = fp_pool.tile([P, RC], F32, tag="fp")
                    for k in range(KT):
                        for h in range(2):
                            nc.tensor.matmul(
                                fp[:, h * 512 : (h + 1) * 512],
                                wqkvT[k][:, m * P : (m + 1) * P],
                                hids[k][:, h * 512 : (h + 1) * 512],
                                start=(k == 0),
                                stop=(k == KT - 1),
                            )
                    nc.vector.tensor_scalar(
                        fusedT[m][:, rc * RC : (rc + 1) * RC],
                        fp[:],
                        bq_sb[:, m : m + 1],
                        None,
                        op0=ALU.add,
                    )

        # ====== Phase 2: attention per (slot, b); heavy slot first ======
        with tc.tile_pool(name="expp", bufs=1) as exp_pool, tc.tile_pool(
            name="vaug", bufs=2
        ) as v_pool, tc.tile_pool(name="nrm", bufs=4) as nrm_pool, tc.tile_pool(
            name="scp", bufs=2, space="PSUM"
        ) as sc_pool, tc.tile_pool(
            name="cxp", bufs=2, space="PSUM"
        ) as cx_pool, tc.tile_pool(
            name="tpp", bufs=1, space="PSUM"
        ) as tp_pool, tc.tile_pool(
            name="ctp", bufs=1, space="PSUM"
        ) as ctp_pool:
            expT = [
                exp_pool.tile(
                    [P, (QT - kt) * P], BF16, tag=f"expT{kt}", name=f"expT{kt}"
                )
                for kt in range(QT)
            ]
            for s in range(2):
                D = d_pair[s]
                for b in range(cfg.B):
                    base = b * cfg.S
                    # v natural + ones column, interleaved [v(128)|1|pad] per kt
                    v_aug = v_pool.tile([P, QT * VW], BF16, tag="v_aug")
                    nc.vector.memset(v_aug[:], 1.0)
                    for g4 in range(QT // 4):
                        tp = tp_pool.tile([P, 512], BF16, tag="tp")
                        for j in range(4):
                            kt = 4 * g4 + j
                            nc.tensor.transpose(
                                tp[:, j * P : (j + 1) * P],
                                vT(s)[:, base + kt * P : base + (kt + 1) * P],
                                ident[:],
                            )
                        for j in range(4):
                            kt = 4 * g4 + j
                            nc.vector.tensor_copy(
                                v_aug[:, kt * VW : kt * VW + P],
                                tp[:, j * P : (j + 1) * P],
                            )
                    # scoresT[k, q] = K^T Q, exp, * alibi-decay F
                    for kt in range(QT):
                        cols = min(D + 1, QT - kt) * P
                        q0 = base + kt * P
                        for c0 in range(0, cols, 1024):
                            cw = min(1024, cols - c0)
                            sc = sc_pool.tile([P, 1024], F32, tag="sc")
                            for n0 in range(0, cw, 512):
                                nw = min(512, cw - n0)
                                nc.tensor.matmul(
                                    sc[:, n0 : n0 + nw],
                                    kTt(s)[:, base + kt * P : base + (kt + 1) * P],
                                    qT(s)[:, q0 + c0 + n0 : q0 + c0 + n0 + nw],
                                    start=True,
                                    stop=True,
                                )
                            nc.scalar.activation(
                                expT[kt][:, c0 : c0 + cw], sc[:, :cw], AF.Exp
                            )
                            nc.vector.tensor_tensor(
                                expT[kt][:, c0 : c0 + cw],
                                expT[kt][:, c0 : c0 + cw],
                                fcat_sb[s][:, c0 : c0 + cw],
                                op=ALU.mult,
                            )
                    # PV with ones-augmented V: ctx natural [q, hd] + den col
                    for t in range(QT):
                        kt0 = max(0, t - D)
                        cx = cx_pool.tile([P, VW], F32, tag="cx")
                        for kt in range(kt0, t + 1):
                            nc.tensor.matmul(
                                cx[:, 0 : P + 1],
                                expT[kt][:, (t - kt) * P : (t - kt + 1) * P],
                                v_aug[:, kt * VW : kt * VW + P + 1],
                                start=(kt == kt0),
                                stop=(kt == t),
                            )
                        rden = nrm_pool.tile([P, 1], F32, tag="rden")
                        nc.vector.reciprocal(rden[:], cx[:, P : P + 1])
                        ctx_n = nrm_pool.tile([P, P], BF16, tag="ctx_n")
                        nc.vector.tensor_scalar(
                            ctx_n[:], cx[:, 0:P], rden[:], None, op0=ALU.mult
                        )
                        ctp = ctp_pool.tile([P, P], BF16, tag="ctp")
                        nc.tensor.transpose(ctp[:], ctx_n[:], ident[:])
                        nc.vector.tensor_copy(
                            ctxT[s][:, base + t * P : base + (t + 1) * P], ctp[:]
                        )
                if b == cfg.B - 1:
                    for j in range(cfg.n_cores):
                        nc.sync.dma_start(
                            a2a_in[s][j],
                            ctxT[s][:, j * cfg.shard : (j + 1) * cfg.shard],
                        )
                    nc.gpsimd.collective_compute(
                        "AllToAll",
                        ALU.bypass,
                        replica_groups=[list(range(cfg.n_cores))],
                        ins=[a2a_in[s].opt()],
                        outs=[a2a_out[s].opt()],
                    )

        # free the qkv/fused space before the dense-phase pools open
        fused_ctx.close()

        # ====== Phase 3: dense + residual (sequence-sharded), two passes ======
        with tc.tile_pool(name="wdl", bufs=1) as wdl_pool, tc.tile_pool(
            name="resp", bufs=1
        ) as res_pool, tc.tile_pool(name="dA", bufs=1) as dA_pool, tc.tile_pool(
            name="ctxf", bufs=1
        ) as ctxf_pool, tc.tile_pool(name="osb", bufs=2) as osb_pool, tc.tile_pool(
            name="dpp", bufs=2, space="PSUM"
        ) as dp_pool:
            ctxf = {}

            def load_ctxf(s):
                for j in range(cfg.n_cores):
                    g = j + 8 * (1 - s)  # slot0 = heads 8..15, slot1 = 0..7
                    t_ = ctxf_pool.tile(
                        [P, cfg.shard], BF16, tag=f"ctxf{g}", name=f"ctxf{g}"
                    )
                    nc.sync.dma_start(t_[:], a2a_out[s][j])
                    ctxf[g] = t_

            load_ctxf(0)  # heavy heads: a2a already landed
            # light-head dense weights + residual load during heavy pass
            for g in range(8):
                wdT[g] = wdl_pool.tile([P, cfg.H], BF16, tag=f"wdTl{g}", name=f"wdTl{g}")
                nc.sync.dma_start(wdT[g][:], wd_d[g * P : (g + 1) * P, :])
            res_sb = []
            for m in range(cfg.shard // P):
                r_ = res_pool.tile([P, cfg.H], F32, tag=f"res{m}", name=f"res{m}")
                nc.sync.dma_start(r_[:], res_d[m * P : (m + 1) * P, :])
                res_sb.append(r_)
            dA = [
                dA_pool.tile([P, cfg.H], F32, tag=f"dA{m}", name=f"dA{m}")
                for m in range(cfg.shard // P)
            ]
            load_ctxf(1)

            # pass 1: heavy heads (a2a already landed) -> dA in SBUF
            for m in range(cfg.shard // P):
                for half in range(2):
                    dp = dp_pool.tile([P, 1024], F32, tag="dpH")
                    for gi, g in enumerate(range(8, 16)):
                        for n0 in range(2):
                            nc.tensor.matmul(
                                dp[:, n0 * 512 : (n0 + 1) * 512],
                                ctxf[g][:, m * P : (m + 1) * P],
                                wdT[g][:, half * 1024 + n0 * 512 : half * 1024 + (n0 + 1) * 512],
                                start=(gi == 0),
                                stop=(gi == 7),
                            )
                    nc.vector.tensor_copy(
                        dA[m][:, half * 1024 : (half + 1) * 1024], dp[:]
                    )
            # pass 2: light heads + dA + residual -> out
            for m in range(cfg.shard // P):
                for half in range(2):
                    dp = dp_pool.tile([P, 1024], F32, tag="dpL")
                    for gi, g in enumerate(range(8)):
                        for n0 in range(2):
                            nc.tensor.matmul(
                                dp[:, n0 * 512 : (n0 + 1) * 512],
                                ctxf[g][:, m * P : (m + 1) * P],
                                wdT[g][:, half * 1024 + n0 * 512 : half * 1024 + (n0 + 1) * 512],
                                start=(gi == 0),
                                stop=(gi == 7),
                            )
                    osb = osb_pool.tile([P, 1024], F32, tag="osb")
                    nc.vector.tensor_tensor(
                        osb[:], dp[:], dA[m][:, half * 1024 : (half + 1) * 1024],
                        op=ALU.add,
                    )
                    nc.vector.tensor_tensor(
                        osb[:], osb[:], res_sb[m][:, half * 1024 : (half + 1) * 1024],
                        op=ALU.add,
                    )
                    nc.sync.dma_start(
                        out_d[m * P : (m + 1) * P, half * 1024 : (half + 1) * 1024],
                        osb[:],
                    )

    nc.compile()
    return nc


def make_in_maps(inputs: dict, cfg: Cfg = DEFAULT_CFG):
    """Shard + pre-transform the full inputs into per-core input maps."""
    hs = np.asarray(inputs["hidden_states"], dtype=np.float32).reshape(cfg.rows, cfg.H)
    hidT = hs.T.astype(BF16NP)  # [H, rows] bf16, shared by all cores
    res = np.asarray(inputs["residual"], dtype=np.float32).reshape(cfg.rows, cfg.H)
    wqkv = np.asarray(inputs["W_qkv"], dtype=np.float32)
    bqkv = np.asarray(inputs["b_qkv"], dtype=np.float32)
    wd = np.asarray(inputs["W_dense"], dtype=np.float32).T.astype(BF16NP)  # [in, out]
    bd = np.asarray(inputs["b_dense"], dtype=np.float32)
    alibi = np.asarray(inputs["alibi"], dtype=np.float32).reshape(cfg.B, cfg.NH, cfg.S)
    slopes = alibi[0, :, 1].astype(np.float64)  # alibi[0, g, k] = slope_g * k
    resb = res + bd[None, :]  # fold dense bias into the residual

    inv_norm = 1.0 / cfg.norm
    QT = cfg.S // P
    pk = np.arange(P, dtype=np.float64)[:, None]
    pq = np.arange(P, dtype=np.float64)[None, :]

    in_maps = []
    for c in range(cfg.n_cores):
        heads = [c + 8, c]  # slot0 = heavy (low slope), slot1 = light
        wsel = np.empty((cfg.wcols, cfg.H), dtype=np.float32)
        bq = np.empty((P, 6), dtype=np.float32)
        fcat = np.zeros((2, P, cfg.S), dtype=np.float64)
        for s, g in enumerate(heads):
            blk = wqkv[g * 384 : (g + 1) * 384]
            wsel[s * 384 : s * 384 + 128] = blk[0:128] * inv_norm
            wsel[s * 384 + 128 : s * 384 + 384] = blk[128:384]
            bq[:, 3 * s + 0] = bqkv[g * 384 : g * 384 + 128] * inv_norm
            bq[:, 3 * s + 1] = bqkv[g * 384 + 128 : g * 384 + 256]
            bq[:, 3 * s + 2] = bqkv[g * 384 + 256 : g * 384 + 384]
            slope = float(slopes[g])
            for d in range(QT):
                f = np.exp(np.minimum(slope * (pk - pq - 128.0 * d), 0.0))
                if d == 0:
                    f = np.triu(f)  # [k, q] layout: k > q (lower tri) -> exactly 0
                fcat[s, :, d * P : (d + 1) * P] = f
        in_maps.append(
            {
                "hidT": hidT,
                "wqkvT": np.ascontiguousarray(wsel.T).astype(BF16NP),
                "bq": bq,
                "fcat": fcat.astype(BF16NP),
                "wd": wd,
                "res": np.ascontiguousarray(resb[c * cfg.shard : (c + 1) * cfg.shard]),
            }
        )
    return in_maps


def assemble_out(results, cfg: Cfg = DEFAULT_CFG) -> np.ndarray:
    out = np.concatenate([results[c]["out"] for c in range(cfg.n_cores)], axis=0)
    return np.ascontiguousarray(out.reshape(cfg.B, cfg.S, cfg.H).astype(np.float32))


_NC_CACHE = {}


def get_nc(d_pair=(15, 6), cfg: Cfg = DEFAULT_CFG):
    key = (d_pair, cfg)
    if key not in _NC_CACHE:
        _NC_CACHE[key] = build_nc(d_pair, cfg)
    return _NC_CACHE[key]


def d_pair_from_inputs(inputs, cfg: Cfg = DEFAULT_CFG):
    alibi = np.asarray(inputs["alibi"], dtype=np.float32).reshape(cfg.B, cfg.NH, cfg.S)
    slopes = alibi[0, :, 1]
    d_heavy = max(slope_to_D(float(s)) for s in slopes[8:16])
    d_light = max(slope_to_D(float(s)) for s in slopes[0:8])
    return (d_heavy, d_light)


def kernel(**inputs) -> np.ndarray:
    from concourse.bass_utils import run_bass_kernel_spmd

    cfg = DEFAULT_CFG
    nc = get_nc(d_pair_from_inputs(inputs, cfg), cfg)
    in_maps = make_in_maps(inputs, cfg)
    r = run_bass_kernel_spmd(nc, in_maps, core_ids=list(range(cfg.n_cores)))
    return assemble_out(r.results, cfg)
